# revision 23
# baseline (speedup 1.0000x reference)
"""Trainium2 Bass kernel for nn_AAttn (area attention block), SPMD over 8 cores.

Reference computation (eval-mode):
  qkv = BN(conv1x1(x, w_qkv))                       # [B,3C,H,W]
  per (batch, area) of B*AREA=8: per head (8, hd=32):
     S = q^T k / sqrt(hd); A = softmax(S, -1); o = v @ A^T
  pe  = BN(dwconv7(v2))
  out = BN(conv1x1(o + pe, w_proj))

Sharding: core i <-> (b, a) = (i//4, i%4) - one area per core (data parallel,
no collectives). Each core gets x rows [16a-3, 16a+19) zero-padded (halo for
the 7x7 depthwise conv), computes its 16 output rows.

Schedule: the ACT-engine exp stream (64 x [128,1024] softmax exps, ~71us
busy) is the hard floor; everything else hides under it:
 - 6 warm-up matmuls on junk data ramp the PE p-state during the input DMAs;
   q/k for both head-groups run right after with ACT drains (ACT is idle
   until the first exp), so the exp stream starts at ~17us.
 - The S->exp pipeline runs S pairs back-to-back per mt; O and the
   ones-matmul denominator D accumulate at lag-2; v/vT and the dwconv
   chains are interleaved as paced fillers from two queues (PE-cost and
   DVE-cost) so neither engine backlogs ahead of normalize/pin ops.
 - dwconv7: 44 taps as bf16 diag matmuls on PE (LDWEIGHTS hides behind the
   stream), 5 taps as DVE STT chains (two interleaved accumulators so the
   serial accumulate dependency never stalls DVE).
 - All PSUM fits the 8 banks: s_ps 2x2 + o 1 + d 1 + dw 2, with early-phase
   q/k/v/vT tiles round-robined through the o/d/dw pools, and the dwconv
   chains two-stage drained (copy to SBUF frees the bank; the on+accd add
   happens later) so bank rotation never crosses a normalize dependency.
 - Output is written bf16 and widened on the host.
"""

import os
import sys
import numpy as np

sys.path.insert(0, "/opt/trn_rl_repo")

import ml_dtypes  # noqa: E402

import concourse.bass as bass  # noqa: E402
from concourse import bacc, mybir  # noqa: E402
from concourse.tile import TileContext  # noqa: E402
from concourse.bass_utils import run_bass_kernel_spmd  # noqa: E402

F32 = mybir.dt.float32
BF16 = mybir.dt.bfloat16

EPS = 1e-5
HEADS = 8
AREA = 4
C = 256
HD = 32          # head dim
B = 2
H = W = 64
ROWS = 16        # output rows per core
HALO = 3
HR = ROWS + 2 * HALO       # 22 halo rows
HWC = W + 2 * HALO         # 70 halo cols
NSP = HR * HWC             # 1540 halo spatial
NCEN = ROWS * W            # 1024 central spatial
G = 2                      # head groups of 4 (128 channels each)

ALL_TAPS = [(dy, dx) for dy in range(7) for dx in range(7)]
NPE = int(os.environ.get("AATTN_PE_TAPS", "44"))
TAPS_PE = ALL_TAPS[:NPE]
TAPS_DVE = ALL_TAPS[NPE:]
NDVE = len(TAPS_DVE)


def build_nc():
    nc = bacc.Bacc("TRN2", target_bir_lowering=False, debug=False, num_devices=8)

    d_x = nc.declare_dram_parameter("x_local", [2, 128, NSP], BF16, isOutput=False)
    d_wq = nc.declare_dram_parameter("wqT", [2, 128, C], BF16, isOutput=False)
    d_wk = nc.declare_dram_parameter("wkT", [2, 128, C], BF16, isOutput=False)
    d_wv = nc.declare_dram_parameter("wvT", [2, 128, C], BF16, isOutput=False)
    d_wp = nc.declare_dram_parameter("wpT", [2, 128, C], BF16, isOutput=False)
    d_bias = nc.declare_dram_parameter("biases", [2, 128, 4], F32, isOutput=False)
    d_mask = nc.declare_dram_parameter("mask", [128, NSP], BF16, isOutput=False)
    d_bvb = nc.declare_dram_parameter("bvb", [128, 2, 8, 128], F32, isOutput=False)
    d_diag = nc.declare_dram_parameter("diag_pe", [128, 2, max(NPE, 1), 128],
                                       BF16, isOutput=False)
    d_wdve = nc.declare_dram_parameter("w_dve", [2, 128, max(NDVE, 1)], F32,
                                       isOutput=False)
    d_out = nc.declare_dram_parameter("out", [2, 128, NCEN], BF16, isOutput=True)

    with TileContext(nc) as tc:
        _build_body(nc, tc, d_x, d_wq, d_wk, d_wv, d_wp, d_bias, d_mask,
                    d_bvb, d_diag, d_wdve, d_out)

    nc.compile()
    return nc


def _build_body(nc, tc, d_x, d_wq, d_wk, d_wv, d_wp, d_bias, d_mask,
                d_bvb, d_diag, d_wdve, d_out):
    from contextlib import ExitStack

    ctx = ExitStack()
    with ctx:
        persist = ctx.enter_context(tc.tile_pool(name="persist", bufs=1))

        # ---- static SBUF tensors ----
        x_sb = persist.tile([128, 2, NSP], BF16, tag="x")
        wq_sb = persist.tile([128, 2, C], BF16, tag="wq")
        wk_sb = persist.tile([128, 2, C], BF16, tag="wk")
        wv_sb = persist.tile([128, 2, C], BF16, tag="wv")
        wp_sb = persist.tile([128, 2, C], BF16, tag="wp")
        b_sb = persist.tile([128, 2, 4], F32, tag="bias")
        mask_sb = persist.tile([128, NSP], BF16, tag="mask")
        bvb_sb = persist.tile([128, 2, 8, 128], F32, tag="bvb")
        diag_sb = persist.tile([128, 2, max(NPE, 1), 128], BF16, tag="diag")
        wdve_sb = persist.tile([128, 2, max(NDVE, 1)], F32, tag="wdve")
        ones_sb = persist.tile([128, 32], BF16, tag="ones")
        junk_sb = persist.tile([128, 512], BF16, tag="junk")

        q_sb = persist.tile([128, 2, NCEN], BF16, tag="q")
        k_sb = persist.tile([128, 2, NCEN], BF16, tag="k")
        xcen_sb = persist.tile([128, 2, NCEN], BF16, tag="xcen")
        vbf_sb = persist.tile([128, 2, NSP], BF16, tag="vbf")
        vT_sb = persist.tile([128, 2, 8, 128], BF16, tag="vT")
        accd_sb = persist.tile([128, 2, NCEN], BF16, tag="accd")
        pesb_sb = persist.tile([128, 2, NCEN], BF16, tag="pesb")
        on_sb = persist.tile([128, 2, NCEN], BF16, tag="on")
        pin_sb = persist.tile([128, 2, NCEN], BF16, tag="pin")
        out_sb = persist.tile([128, 2, NCEN], BF16, tag="outsb")

        # ---- input DMAs, most-urgent first ----
        for t in range(2):
            nc.sync.dma_start(out=x_sb[:, t, :], in_=d_x[t])
        for t in range(2):
            nc.sync.dma_start(out=wq_sb[:, t, :], in_=d_wq[t])
        for t in range(2):
            nc.sync.dma_start(out=wk_sb[:, t, :], in_=d_wk[t])
        for t in range(2):
            nc.sync.dma_start(out=b_sb[:, t, :], in_=d_bias[t])
        for t in range(2):
            nc.sync.dma_start(out=wv_sb[:, t, :], in_=d_wv[t])
        nc.sync.dma_start(out=mask_sb[:], in_=d_mask[:])
        nc.sync.dma_start(out=bvb_sb[:], in_=d_bvb[:])
        for t in range(2):
            nc.sync.dma_start(out=wp_sb[:, t, :], in_=d_wp[t])
        if NPE:
            nc.sync.dma_start(out=diag_sb[:], in_=d_diag[:])
        if NDVE:
            for t in range(2):
                nc.sync.dma_start(out=wdve_sb[:, t, :], in_=d_wdve[t])

        nc.gpsimd.memset(junk_sb[:], 1.0)
        nc.vector.memset(ones_sb[:], 1.0)

        def x3(t):
            return x_sb[:, t, :].rearrange("p (r c) -> p r c", c=HWC)

        def cen(ap3, q8):  # 512-col central chunk (8 rows) of a [p, 22, 70] view
            return ap3[:, HALO + 8 * q8: HALO + 8 * (q8 + 1), HALO: HALO + W]

        def vbf3(t):
            return vbf_sb[:, t, :].rearrange("p (r c) -> p r c", c=HWC)

        for t in range(2):
            nc.vector.tensor_copy(
                xcen_sb[:, t, :],
                x3(t)[:, HALO: HALO + ROWS, HALO: HALO + W])

        ps_s = ctx.enter_context(tc.tile_pool(name="ps_s", bufs=2, space="PSUM"))
        ps_o = ctx.enter_context(tc.tile_pool(name="ps_o", bufs=1, space="PSUM"))
        ps_d = ctx.enter_context(tc.tile_pool(name="ps_d", bufs=1, space="PSUM"))
        ps_dw = ctx.enter_context(tc.tile_pool(name="ps_dw", bufs=2, space="PSUM"))
        at_pool = ctx.enter_context(tc.tile_pool(name="at", bufs=16))
        small = ctx.enter_context(tc.tile_pool(name="small", bufs=4))

        # early-phase psum round-robin: o, d and the two dw banks
        _early = [ps_o, ps_d, ps_dw, ps_dw]
        _eidx = [0]

        def early_tile(free=512):
            pool = _early[_eidx[0] % 4]
            _eidx[0] += 1
            return pool.tile([128, free], F32, tag=pool.name, name="early_ps")

        # ---------- emission helpers ----------
        def emit_warmup(n):
            psum = ps_dw.tile([128, 512], F32, tag="ps_dw", name="warm_ps")
            for i in range(n):
                nc.tensor.matmul(psum[:], lhsT=junk_sb[:, 0:128],
                                 rhs=junk_sb[:], start=True, stop=True)

        def emit_qk(w_t, o_t, bidx, g, ch, act=True):
            """one [128,512] chunk of q or k: 2 matmuls + bias drain.  The
            g=0 chunks drain on ACT (idle before the exp stream); the g=1
            filler chunks drain on DVE so they don't intrude on the exps."""
            psum = early_tile()
            for kt in range(2):
                nc.tensor.matmul(
                    psum[:],
                    lhsT=(w_t[:, kt, 128 * g: 128 * (g + 1)]),
                    rhs=(cen(x3(kt), ch)),
                    start=(kt == 0), stop=(kt == 1))
            if act:
                nc.scalar.add(
                    o_t[:, g, 512 * ch: 512 * (ch + 1)], psum[:],
                    b_sb[:, g, bidx: bidx + 1])
            else:
                nc.vector.tensor_scalar(
                    out=o_t[:, g, 512 * ch: 512 * (ch + 1)], in0=psum[:],
                    scalar1=b_sb[:, g, bidx: bidx + 1], scalar2=None,
                    op0=mybir.AluOpType.add)
                dve_ns[0] += 720

        V_CHUNKS = ((0, 512), (512, 512), (1024, 512), (1536, 4))

        def emit_v(g, ci):
            """one chunk of halo v -> vbf bf16: (psum + bv) * mask."""
            c0, cn = V_CHUNKS[ci]
            psum = early_tile(cn)
            for kt in range(2):
                nc.tensor.matmul(
                    psum[:],
                    lhsT=(wv_sb[:, kt, 128 * g: 128 * (g + 1)]),
                    rhs=(x_sb[:, kt, c0: c0 + cn]),
                    start=(kt == 0), stop=(kt == 1))
            nc.vector.scalar_tensor_tensor(
                out=vbf_sb[:, g, c0: c0 + cn], in0=psum[:],
                scalar=b_sb[:, g, 2: 3], in1=mask_sb[:, c0: c0 + cn],
                op0=mybir.AluOpType.add, op1=mybir.AluOpType.mult)
            dve_ns[0] += 760

        def emit_vT(g, half):
            """vT[m, d4] for group g, mt chunks [4*half, 4*half+4)."""
            pst = early_tile()
            pst4 = pst.rearrange("p (mt c) -> p mt c", c=128)
            for i in range(4):
                mt = 4 * half + i
                for kt in range(2):
                    nc.tensor.matmul(
                        pst4[:, i, :],
                        lhsT=xcen_sb[:, kt, 128 * mt: 128 * (mt + 1)],
                        rhs=wv_sb[:, kt, 128 * g: 128 * (g + 1)],
                        start=(kt == 0), stop=(kt == 1))
            nc.vector.scalar_tensor_tensor(
                out=vT_sb[:, g, 4 * half: 4 * half + 4, :], in0=pst[:],
                scalar=1.0, in1=bvb_sb[:, g, 4 * half: 4 * half + 4, :],
                op0=mybir.AluOpType.bypass, op1=mybir.AluOpType.add)
            dve_ns[0] += 760

        # dwconv: PE diag chains per (t, ch) + DVE STT chains per (t, ch)
        def dw_win(t, dy, dx, ch):
            return vbf3(t)[:, dy + 8 * ch: dy + 8 * ch + 8, dx: dx + 64]

        dw_ps = {}
        dve_seeded = {}

        def start_dw(t, ch):
            dw_ps[(t, ch)] = ps_dw.tile([128, 512], F32, tag="ps_dw", name="dw_psum")

        def pe_taps(t, ch, j0, j1):
            for j in range(j0, j1):
                dy, dx = TAPS_PE[j]
                nc.tensor.matmul(
                    dw_ps[(t, ch)][:], lhsT=diag_sb[:, t, j, :],
                    rhs=dw_win(t, dy, dx, ch),
                    start=(j == 0), stop=(j == NPE - 1))

        def dve_tap(t, j):
            dy, dx = TAPS_DVE[j]
            win = vbf3(t)[:, dy: dy + ROWS, dx: dx + 64]
            if not dve_seeded.get(t):
                dve_seeded[t] = True
                nc.vector.tensor_scalar(
                    out=accd_sb[:, t, :], in0=win,
                    scalar1=wdve_sb[:, t, j: j + 1], scalar2=None,
                    op0=mybir.AluOpType.mult)
            else:
                nc.vector.scalar_tensor_tensor(
                    out=accd_sb[:, t, :], in0=win,
                    scalar=wdve_sb[:, t, j: j + 1], in1=accd_sb[:, t, :],
                    op0=mybir.AluOpType.mult, op1=mybir.AluOpType.add)
            dve_ns[0] += 1280

        def emit_pin1(t, ch):
            """stage 1: pesb = copy(pe_ps) - frees the dw psum bank, no deps."""
            sl = slice(512 * ch, 512 * (ch + 1))
            if NPE:
                nc.vector.tensor_copy(pesb_sb[:, t, sl], dw_ps[(t, ch)][:])
                dve_ns[0] += 720

        def emit_pin2(t, ch):
            """stage 2: pin = on + pesb (+ accd)."""
            sl = slice(512 * ch, 512 * (ch + 1))
            cur = on_sb[:, t, sl]
            if NDVE:
                t1 = small.tile([128, 512], BF16, tag="tmpc", name="t1")
                nc.vector.tensor_tensor(
                    out=t1[:], in0=cur, in1=accd_sb[:, t, sl],
                    op=mybir.AluOpType.add)
                cur = t1[:]
            if NPE:
                nc.vector.tensor_tensor(
                    out=pin_sb[:, t, sl], in0=cur, in1=pesb_sb[:, t, sl],
                    op=mybir.AluOpType.add)
            else:
                nc.vector.tensor_copy(pin_sb[:, t, sl], cur)

        def emit_proj(ch, o):
            psum = ps_dw.tile([128, 512], F32, tag="ps_dw", name="pj_ps")
            for t in range(2):
                nc.tensor.matmul(
                    psum[:],
                    lhsT=wp_sb[:, t, 128 * o: 128 * (o + 1)],
                    rhs=pin_sb[:, t, 512 * ch: 512 * (ch + 1)],
                    start=(t == 0), stop=(t == 1))
            nc.scalar.add(
                out_sb[:, o, 512 * ch: 512 * (ch + 1)], psum[:],
                b_sb[:, o, 3: 4])
            nc.sync.dma_start(
                out=d_out[o, :, 512 * ch: 512 * (ch + 1)],
                in_=out_sb[:, o, 512 * ch: 512 * (ch + 1)])

        # ---------- attention machinery ----------
        ats = {}

        def s_and_exp(g, ch, mt, half):
            s_ps = ps_s.tile([128, 2, 512], F32, tag="s", name="s_ps")
            for il in range(2):
                i = 2 * half + il
                nc.tensor.matmul(
                    s_ps[:, il, :],
                    lhsT=k_sb[32 * i: 32 * (i + 1), g,
                              128 * mt: 128 * (mt + 1)],
                    rhs=q_sb[32 * i: 32 * (i + 1), g,
                             512 * ch: 512 * (ch + 1)],
                    start=True, stop=True, skip_group_check=True,
                    tile_position=(32 * i, 0))
            at = at_pool.tile([128, 2, 512], BF16, tag="at", name="at")
            nc.scalar.activation(at[:], s_ps[:],
                                 mybir.ActivationFunctionType.Exp)
            ats[(mt, half)] = at

        def od_mt(g, o_ps, d_ps, mt):
            for half in range(2):
                at = ats[(mt, half)]
                for il in range(2):
                    i = 2 * half + il
                    nc.tensor.matmul(
                        o_ps[32 * i: 32 * (i + 1), :],
                        lhsT=vT_sb[:, g, mt, 32 * i: 32 * (i + 1)],
                        rhs=at[:, il, :],
                        start=(mt == 0), stop=(mt == 7),
                        skip_group_check=True,
                        tile_position=(0, 32 * i))
                    nc.tensor.matmul(
                        d_ps[32 * i: 32 * (i + 1), :],
                        lhsT=ones_sb[:, 0:32],
                        rhs=at[:, il, :],
                        start=(mt == 0), stop=(mt == 7),
                        skip_group_check=True,
                        tile_position=(0, 32 * i))

        def normalize(g, ch, o_ps, d_ps):
            r_sb = small.tile([128, 512], F32, tag="r", name="r_sb")
            nc.vector.reciprocal_approx_fast(out=r_sb[:], in_=d_ps[:])
            nc.vector.scalar_tensor_tensor(
                out=on_sb[:, g, 512 * ch: 512 * (ch + 1)],
                in0=o_ps[:], scalar=1.0, in1=r_sb[:],
                op0=mybir.AluOpType.bypass, op1=mybir.AluOpType.mult)
            dve_ns[0] += 1500

        # ---------- two filler queues (PE-cost and DVE-cost items) ----------
        peq = []
        dveq = []
        pe_mark = {}
        pe_gates = {}
        dve_ns = [0]

        def fpe(est, fn, *a):
            peq.append((est, lambda: fn(*a)))

        def fdve(est, fn, *a):
            dveq.append((est, lambda: fn(*a)))

        fpe(600, emit_qk, wk_sb, k_sb, 1, 0, 1, False)
        fpe(600, emit_qk, wq_sb, q_sb, 0, 0, 1, False)
        fpe(600, emit_qk, wq_sb, q_sb, 0, 1, 0, False)
        fpe(600, emit_qk, wq_sb, q_sb, 0, 1, 1, False)
        fpe(600, emit_qk, wk_sb, k_sb, 1, 1, 0, False)
        fpe(600, emit_qk, wk_sb, k_sb, 1, 1, 1, False)
        pe_mark["qk_done"] = len(peq)
        fpe(1100, emit_vT, 0, 0)
        fpe(1100, emit_vT, 0, 1)
        pe_mark["vt0_done"] = len(peq)
        for g in range(2):
            for ci in range(3):
                fpe(600, emit_v, g, ci)
            fpe(100, emit_v, g, 3)
        pe_mark["v_done"] = len(peq)
        fpe(1100, emit_vT, 1, 0)
        fpe(1100, emit_vT, 1, 1)
        pe_mark["vt1_done"] = len(peq)
        norm_done = set()
        CHAINS = ((0, 0), (0, 1), (1, 0), (1, 1))
        for ci, (t, ch) in enumerate(CHAINS):
            if NPE:
                fpe(0, start_dw, t, ch)
                for j0 in range(0, NPE, 2):
                    j1 = min(j0 + 2, NPE)
                    fpe(300 * (j1 - j0), pe_taps, t, ch, j0, j1)
                fpe(0, emit_pin1, t, ch)
            pe_mark[f"chain{t}{ch}"] = len(peq)
        # DVE taps: alternate the two accd chains so the serial accumulate
        # dependency never stalls the engine
        for j in range(NDVE):
            for t in range(2):
                fdve(1300, dve_tap, t, j)
        _pi = [0]
        _di = [0]

        def pump(mt_key, pe_budget=1600):
            while _pi[0] < len(peq) and pe_budget > 0:
                gate = pe_gates.get(_pi[0])
                if gate is not None and gate not in norm_done:
                    break
                est, fn = peq[_pi[0]]
                _pi[0] += 1
                fn()
                pe_budget -= est
            gi, mt = mt_key
            # DVE taps: pop only while cumulative DVE load stays ~2us behind
            # the wall-clock estimate, so normalize/pin never queue deep
            wall = (8 * gi + mt) * 2230
            npop = 0
            if _pi[0] >= pe_mark["v_done"]:
                while (_di[0] < len(dveq) and npop < 2
                       and dve_ns[0] + 1280 < wall - 2000):
                    est, fn = dveq[_di[0]]
                    _di[0] += 1
                    fn()
                    npop += 1

        def drain_pe_to(mark):
            while _pi[0] < pe_mark[mark]:
                est, fn = peq[_pi[0]]
                _pi[0] += 1
                fn()

        def drain_dve_to(idx):
            while _di[0] < idx:
                est, fn = dveq[_di[0]]
                _di[0] += 1
                fn()

        # ---------- the schedule ----------
        emit_warmup(5)
        emit_qk(wq_sb, q_sb, 0, 0, 0)
        emit_qk(wk_sb, k_sb, 1, 0, 0)

        GROUPS = [(0, 0), (0, 1), (1, 0), (1, 1)]
        o_ps = d_ps = None
        prev = None     # (o_ps, d_ps, g, ch)

        for gi, (g, ch) in enumerate(GROUPS):
            prev, o_ps, d_ps = ((o_ps, d_ps) + GROUPS[gi - 1] if gi else None,
                                ps_o.tile([128, 512], F32, tag="ps_o", name="o_ps"),
                                ps_d.tile([128, 512], F32, tag="ps_d", name="d_ps"))
            for mt in range(8):
                s_and_exp(g, ch, mt, 0)
                s_and_exp(g, ch, mt, 1)
                if prev is not None and mt < 2:
                    od_mt(prev[2], prev[0], prev[1], 6 + mt)
                    if mt == 1:
                        normalize(prev[2], prev[3], prev[0], prev[1])
                        norm_done.add((prev[2], prev[3]))
                if mt == 3 and gi == 0:
                    drain_pe_to("qk_done")
                if mt == 4 and gi == 0:
                    drain_pe_to("vt0_done")
                if gi == 3 and mt == 5:
                    drain_pe_to("chain10")
                    drain_dve_to(len(dveq))
                    emit_pin2(0, 0)
                    emit_pin2(1, 0)
                    emit_pin2(0, 1)
                    emit_proj(0, 0)
                    emit_proj(0, 1)
                if gi == 0:
                    if mt >= 5:
                        od_mt(g, o_ps, d_ps, 2 * (mt - 5))
                        od_mt(g, o_ps, d_ps, 2 * (mt - 5) + 1)
                elif 2 <= mt:
                    od_mt(g, o_ps, d_ps, mt - 2)
                if (gi, mt) != (0, 0):
                    pump((gi, mt))

        # tail: finish group (1,1)
        od_mt(1, o_ps, d_ps, 6)
        od_mt(1, o_ps, d_ps, 7)
        normalize(1, 1, o_ps, d_ps)
        norm_done.add((1, 1))
        drain_pe_to("chain11")
        drain_dve_to(len(dveq))
        emit_pin2(1, 1)
        emit_proj(1, 0)
        emit_proj(1, 1)


# ---------------------------------------------------------------------------
# host side
# ---------------------------------------------------------------------------

_NC_CACHE = {}


def _get_nc():
    if "nc" not in _NC_CACHE:
        _NC_CACHE["nc"] = build_nc()
    return _NC_CACHE["nc"]


def _prep_shared(w_qkv, g_qkv, b_qkv, m_qkv, var_qkv,
                 w_pe, g_pe, b_pe, m_pe, var_pe,
                 w_proj, g_proj, b_proj, m_proj, var_proj):
    f32 = np.float32
    bf = ml_dtypes.bfloat16
    s_qkv = (g_qkv / np.sqrt(var_qkv + EPS)).astype(f32)
    Wall = (w_qkv * s_qkv[:, None]).astype(f32)
    ball = (b_qkv - m_qkv * s_qkv).astype(f32)

    hs = np.arange(HEADS)
    perm_q = (hs[:, None] * 96 + np.arange(HD)[None, :]).reshape(-1)
    perm_k = perm_q + HD
    perm_v = perm_q + 2 * HD

    sc = f32(1.0 / np.sqrt(HD))
    Wq = Wall[perm_q] * sc
    bq = ball[perm_q] * sc
    Wk = Wall[perm_k]
    bk = ball[perm_k]
    Wv = Wall[perm_v]
    bv = ball[perm_v]

    s_pe = (g_pe / np.sqrt(var_pe + EPS)).astype(f32)
    wpe = (w_pe[:, 0] * s_pe[:, None, None]).astype(f32)      # [256,7,7]
    bpe = (b_pe - m_pe * s_pe).astype(f32)

    s_p = (g_proj / np.sqrt(var_proj + EPS)).astype(f32)
    Wp = (w_proj * s_p[:, None]).astype(f32)                  # [256(o),256(c)]
    bp = (b_proj - m_proj * s_p + Wp @ bpe).astype(f32)

    def kt(wT):  # [256(c_in),256(c_out)] -> [2,128,256]
        return np.ascontiguousarray(wT.reshape(2, 128, wT.shape[1]))

    d = {
        "wqT": kt(Wq.T).astype(bf),
        "wkT": kt(Wk.T).astype(bf),
        "wvT": kt(Wv.T).astype(bf),
        "wpT": kt(Wp.T).astype(bf),
    }
    biases = np.zeros((2, 128, 4), f32)
    for t in range(2):
        biases[t, :, 0] = bq[128 * t: 128 * (t + 1)]
        biases[t, :, 1] = bk[128 * t: 128 * (t + 1)]
        biases[t, :, 2] = bv[128 * t: 128 * (t + 1)]
        biases[t, :, 3] = bp[128 * t: 128 * (t + 1)]
    d["biases"] = biases

    bvb = np.zeros((128, 2, 8, 128), f32)
    for g in range(2):
        bvb[:, g, :, :] = bv[128 * g: 128 * (g + 1)][None, None, :]
    d["bvb"] = bvb

    wpe_flat = wpe.reshape(256, 49)
    cc = np.arange(128)
    diag = np.zeros((128, 2, max(NPE, 1), 128), f32)
    for j, (dy, dx) in enumerate(TAPS_PE):
        tap = dy * 7 + dx
        for t in range(2):
            diag[cc, t, j, cc] = wpe_flat[128 * t + cc, tap]
    d["diag_pe"] = diag.astype(bf)

    wdve = np.zeros((2, 128, max(NDVE, 1)), f32)
    for j, (dy, dx) in enumerate(TAPS_DVE):
        tap = dy * 7 + dx
        for t in range(2):
            wdve[t, :, j] = wpe_flat[128 * t: 128 * (t + 1), tap]
    d["w_dve"] = wdve
    return d


def _prep_core(x, core):
    b, a = divmod(core, 4)
    f32 = np.float32
    xl = np.zeros((C, HR, HWC), f32)
    r0 = 16 * a - HALO
    r1 = 16 * a + ROWS + HALO
    s0, s1 = max(r0, 0), min(r1, H)
    xl[:, s0 - r0: s1 - r0, HALO: HALO + W] = x[b, :, s0:s1, :]

    mask = np.zeros((HR, HWC), f32)
    mask[s0 - r0: s1 - r0, HALO: HALO + W] = 1.0
    mask_dense = np.ascontiguousarray(
        np.broadcast_to(mask.reshape(1, NSP), (128, NSP)))
    return (np.ascontiguousarray(xl.reshape(2, 128, NSP)).astype(ml_dtypes.bfloat16),
            mask_dense.astype(ml_dtypes.bfloat16))


def kernel(**inputs):
    x = np.asarray(inputs["x"], np.float32)
    shared = _prep_shared(
        *[np.asarray(inputs[k], np.float32) for k in (
            "w_qkv", "g_qkv", "b_qkv", "m_qkv", "var_qkv",
            "w_pe", "g_pe", "b_pe", "m_pe", "var_pe",
            "w_proj", "g_proj", "b_proj", "m_proj", "var_proj")])

    in_maps = []
    for core in range(8):
        xl, mask = _prep_core(x, core)
        m = dict(shared)
        m["x_local"] = xl
        m["mask"] = mask
        in_maps.append(m)

    nc = _get_nc()
    outs = None
    last_exc = None
    for _attempt in range(3):
        try:
            res = run_bass_kernel_spmd(nc, in_maps, core_ids=list(range(8)))
            outs = res.results
            break
        except Exception as e:  # intermittent device errors: retry
            last_exc = e
            import time
            time.sleep(3)
    if outs is None:
        raise last_exc

    y = np.zeros((B, C, H, W), np.float32)
    for core in range(8):
        b, a = divmod(core, 4)
        o = np.asarray(outs[core]["out"], np.float32).reshape(C, ROWS, W)
        y[b, :, 16 * a: 16 * a + ROWS, :] = o
    return y


if __name__ == "__main__":
    rng = np.random.default_rng(0)
    ins = {
        "x": rng.standard_normal((2, C, H, W)).astype(np.float32),
        "w_qkv": (rng.standard_normal((768, 256)) * 0.05).astype(np.float32),
        "g_qkv": rng.uniform(size=768).astype(np.float32),
        "b_qkv": (rng.standard_normal(768) * 0.05).astype(np.float32),
        "m_qkv": (rng.standard_normal(768) * 0.05).astype(np.float32),
        "var_qkv": rng.uniform(size=768).astype(np.float32),
        "w_pe": (rng.standard_normal((256, 1, 7, 7)) * 0.05).astype(np.float32),
        "g_pe": rng.uniform(size=256).astype(np.float32),
        "b_pe": (rng.standard_normal(256) * 0.05).astype(np.float32),
        "m_pe": (rng.standard_normal(256) * 0.05).astype(np.float32),
        "var_pe": rng.uniform(size=256).astype(np.float32),
        "w_proj": (rng.standard_normal((256, 256)) * 0.05).astype(np.float32),
        "g_proj": rng.uniform(size=256).astype(np.float32),
        "b_proj": (rng.standard_normal(256) * 0.05).astype(np.float32),
        "m_proj": (rng.standard_normal(256) * 0.05).astype(np.float32),
        "var_proj": rng.uniform(size=256).astype(np.float32),
    }
    y = kernel(**ins)
    print("kernel ran, out shape", y.shape, "absmax", np.abs(y).max())
